# revision 11
# baseline (speedup 1.0000x reference)
"""Trainium2 Bass kernel for a dense transformer block (pre-LN, causal, RoPE).

Sharding: data-parallel over batch. B=8 batch elements, 8 NeuronCores, one
batch element per core; weights replicated. No collectives needed.

All-transposed dataflow (token axis T on the free dim everywhere) so no
on-device transposes are needed except Q/K (PE-transpose):
  xT [D, T] bf16 in; LN stats via ones-matmul (mean & E[x^2] land PSUM-
  replicated across partitions); znT = (xT - mu) * rstd directly [d, t].
  Q/K = znT.T @ wq/wk in [t, ch], RoPE (host tables, hs cols permuted
  even|odd, HS**-0.5 folded into q tables), PE-transpose -> QT/KT [ch, t].
  V = znT.T @ wv stays [t, ch].
  attention in head PAIRS via tile_position col-tiling; scoresT pairs share
  a 2-bank PSUM tile (one exp per [128,1024] on ACT); causal 0/1 masks
  multiply on GPSIMD; ones-matmuls write softmax denominators; normalize
  via exp(-ln(d)) (single ACT table set for the whole kernel - the
  act-table monkey-patch forces exp+ln onto natural_log_exp_and_others).
  proj outputs transposed [d, t] (lhsT = w_proj rows) + xT residual ->
  x2T; LN2 same as LN1; FFN1 -> hT [f, t] with relu+bias on DVE; FFN2
  outputs [d, t] (lhsT = w2 cols) + x2T residual -> out [D, T]; host
  transposes back to [T, D].  LN affine folded into weights host-side;
  bias matmuls are emitted only when some folded bias is nonzero.
"""

import os
import sys
import numpy as np

for _p in ("/opt/trn_rl_repo", "/root/.axon_site/_ro/trn_rl_repo"):
    if os.path.isdir(_p) and _p not in sys.path:
        sys.path.append(_p)

import ml_dtypes

import concourse.bass as bass
import concourse.tile as tile
from concourse import bacc, mybir
from concourse.bass import ts
from concourse.bass_utils import run_bass_kernel_spmd

BF16 = mybir.dt.bfloat16
F32 = mybir.dt.float32
AF = mybir.ActivationFunctionType
ALU = mybir.AluOpType

B, T, D, H, HS, F = 8, 1024, 1024, 16, 64, 4096
NT = T // 128   # 8 T-tiles
ND = D // 128   # 8 D-chunks
NF = F // 128   # 32 F-chunks
NCORES = 8


def _bcast_heads(ap2d, nheads=H):
    """[128, J] AP -> [128, nheads, J] broadcast along a step-0 middle dim."""
    return bass.AP(ap2d.tensor, ap2d.offset, [ap2d.ap[0], [0, nheads], ap2d.ap[-1]])


def _patch_act_tables():
    """Force Exp and Ln onto the combined natural_log_exp_and_others set so
    the whole kernel runs on ONE resident ACT table (no 1.3us reloads)."""
    from concourse import hw_specs, bacc as _bacc
    if getattr(hw_specs, "_act_tables_patched", False):
        return
    orig = hw_specs.get_activation_tables

    def patched(arch):
        t = orig(arch)
        if "natural_log_exp_and_others" in t:
            for name in ("exp_and_others", "natural_log", "exp_and_friends"):
                if name in t:
                    t[name] = set()
        return t

    hw_specs.get_activation_tables = patched
    _bacc.get_activation_tables = patched
    hw_specs._act_tables_patched = True


def build_kernel(with_bias=False):
    import contextlib

    _patch_act_tables()
    nc = bacc.Bacc("TRN2", target_bir_lowering=False, debug=False,
                   num_devices=NCORES)

    # ---- external I/O ------------------------------------------------------
    xt_d = nc.dram_tensor("xt", [128, ND, T], BF16, kind="ExternalInput")
    wq_d = nc.dram_tensor("wq", [128, ND, D], BF16, kind="ExternalInput")
    wk_d = nc.dram_tensor("wk", [128, ND, D], BF16, kind="ExternalInput")
    wv_d = nc.dram_tensor("wv", [128, ND, D], BF16, kind="ExternalInput")
    wp_d = nc.dram_tensor("wp", [128, ND, D], BF16, kind="ExternalInput")
    w1_d = nc.dram_tensor("w1", [ND, 128, F], BF16, kind="ExternalInput")
    w2_d = nc.dram_tensor("w2", [NF, 128, D], BF16, kind="ExternalInput")
    rope_d = nc.dram_tensor("rope", [128, NT, 4, HS], BF16, kind="ExternalInput")
    mask_d = nc.dram_tensor("mask", [128, 2, 1024], BF16, kind="ExternalInput")
    ident_d = nc.dram_tensor("ident", [128, 128], BF16, kind="ExternalInput")
    ones64_d = nc.dram_tensor("ones64", [128, 64], BF16, kind="ExternalInput")
    ones128_d = nc.dram_tensor("ones128", [128, 128], BF16, kind="ExternalInput")
    onesrow_d = nc.dram_tensor("onesrow", [1, 512], BF16, kind="ExternalInput")
    brows_d = nc.dram_tensor("brows", [1, 4 * D], BF16, kind="ExternalInput")
    b1t_d = nc.dram_tensor("b1t", [128, NF], F32, kind="ExternalInput")
    out_d = nc.dram_tensor("out", [128, ND, T], F32, kind="ExternalOutput")

    with tile.TileContext(nc) as tc:
        ctx = contextlib.ExitStack()
        with ctx:
            consts = ctx.enter_context(tc.tile_pool(name="consts", bufs=1))
            slabs = ctx.enter_context(tc.tile_pool(name="slabs", bufs=4))
            xpool = ctx.enter_context(tc.tile_pool(name="xpool", bufs=1))
            small = ctx.enter_context(tc.tile_pool(name="small", bufs=1))
            psA = ctx.enter_context(  # 2-bank tiles
                tc.tile_pool(name="psA", bufs=3, space="PSUM"))
            psB = ctx.enter_context(  # 1-bank tiles
                tc.tile_pool(name="psB", bufs=2, space="PSUM"))

            # ---- global constants -----------------------------------------
            ident = consts.tile([128, 128], BF16)
            nc.sync.dma_start(out=ident, in_=ident_d.ap())
            ones64 = consts.tile([128, 64], BF16)
            nc.sync.dma_start(out=ones64, in_=ones64_d.ap())
            ones128 = consts.tile([128, 128], BF16)
            nc.sync.dma_start(out=ones128, in_=ones128_d.ap())
            onesrow = consts.tile([1, 512], BF16)
            nc.sync.dma_start(out=onesrow, in_=onesrow_d.ap())
            b1t = consts.tile([128, NF], F32)
            nc.sync.dma_start(out=b1t, in_=b1t_d.ap())
            eps = consts.tile([128, 1], F32)
            nc.vector.memset(eps, 1e-5)
            brows = None
            if with_bias:
                brows = consts.tile([1, 4 * D], BF16)
                nc.sync.dma_start(out=brows, in_=brows_d.ap())

            # ---- x in (transposed, bf16) ----------------------------------
            xT = xpool.tile([128, ND, T], BF16)
            for xc in range(4):
                nc.sync.dma_start(out=xT[:, 2 * xc:2 * xc + 2, :],
                                  in_=xt_d.ap()[:, 2 * xc:2 * xc + 2, :])

            # ---- layernorm on transposed activations ----------------------
            # mean/E[x^2] via ones-matmul (PSUM-replicated over partitions);
            # rstd = exp(-0.5*ln(var+eps)); apply with two TT ops per chunk.
            def layernorm_T(src, dst):
                """src [128, ND, T] bf16 -> dst [128, ND, T] bf16."""
                mu_ps = psA.tile([128, 1024], F32, tag="A")
                ms_ps = psA.tile([128, 1024], F32, tag="A")
                for c in range(ND):
                    sq = small.tile([128, T], BF16, tag="sq")
                    nc.vector.tensor_mul(out=sq, in0=src[:, c, :],
                                         in1=src[:, c, :])
                    for tb in range(2):
                        tbs = slice(tb * 512, (tb + 1) * 512)
                        nc.tensor.matmul(mu_ps[:, tbs], ones128,
                                         src[:, c, tbs], start=(c == 0),
                                         stop=(c == ND - 1))
                        nc.tensor.matmul(ms_ps[:, tbs], ones128, sq[:, tbs],
                                         start=(c == 0), stop=(c == ND - 1))
                mu = small.tile([128, T], F32, tag="mu")
                nc.vector.tensor_scalar_mul(out=mu, in0=mu_ps,
                                            scalar1=1.0 / D)
                var = small.tile([128, T], F32, tag="var")
                nc.vector.tensor_mul(out=var, in0=mu, in1=mu)  # mu^2
                msd = small.tile([128, T], F32, tag="msd")
                nc.vector.tensor_scalar_mul(out=msd, in0=ms_ps,
                                            scalar1=1.0 / D)
                nc.vector.tensor_sub(out=var, in0=msd, in1=var)
                nc.scalar.activation(out=var, in_=var, func=AF.Ln,
                                     bias=eps, scale=1.0)
                rstd = small.tile([128, T], BF16, tag="rstd")
                nc.scalar.activation(out=rstd, in_=var, func=AF.Exp,
                                     scale=-0.5)
                for c in range(ND):
                    cen = small.tile([128, T], BF16, tag="cen")
                    nc.vector.tensor_sub(out=cen, in0=src[:, c, :], in1=mu)
                    nc.vector.tensor_mul(out=dst[:, c, :], in0=cen, in1=rstd)

            znT = slabs.tile([128, ND, T], BF16, tag="slab")
            layernorm_T(xT, znT)

            QT = slabs.tile([128, ND, T], BF16, tag="slab")
            KT = slabs.tile([128, ND, T], BF16, tag="slab")

            # ============ attention super-phase (scoped pool) ==============
            actx = contextlib.ExitStack()
            with actx:
                apool = actx.enter_context(tc.tile_pool(name="apool", bufs=2))
                ppool = actx.enter_context(tc.tile_pool(name="ppool", bufs=8))

                rope_sb = apool.tile([128, NT, 4, HS], BF16, tag="rope")
                nc.sync.dma_start(out=rope_sb, in_=rope_d.ap())
                mask_sb = apool.tile([128, 2, 1024], BF16, tag="mask")
                nc.sync.dma_start(out=mask_sb, in_=mask_d.ap())

                def qkv_proj(w_dram, brow_idx):
                    w_sb = apool.tile([128, ND, D], BF16, tag="w")
                    for wc in range(4):
                        nc.sync.dma_start(
                            out=w_sb[:, 2 * wc:2 * wc + 2, :],
                            in_=w_dram.ap()[:, 2 * wc:2 * wc + 2, :])
                    for tt in range(NT):
                        ps = psA.tile([128, 1024], F32, tag="A")
                        last = ND - 1
                        for c in range(ND):
                            fin = (c == last and brow_idx is None)
                            lhsT = znT[:, c, ts(tt, 128)]
                            nc.tensor.matmul(ps[:, 0:512], lhsT,
                                             w_sb[:, c, 0:512],
                                             start=(c == 0), stop=fin)
                            nc.tensor.matmul(ps[:, 512:1024], lhsT,
                                             w_sb[:, c, 512:1024],
                                             start=(c == 0), stop=fin)
                        if brow_idx is not None:
                            o = brow_idx * D
                            nc.tensor.matmul(ps[:, 0:512], onesrow[:, 0:128],
                                             brows[0:1, o:o + 512],
                                             start=False, stop=True)
                            nc.tensor.matmul(ps[:, 512:1024],
                                             onesrow[:, 0:128],
                                             brows[0:1, o + 512:o + 1024],
                                             start=False, stop=True)
                        yield tt, ps

                # -- Q then K: copy out of PSUM, rope, PE-transpose
                for w_dram, brow_idx, dstT, tblc, tbls in (
                        (wq_d, 0 if with_bias else None, QT, 0, 1),
                        (wk_d, 1 if with_bias else None, KT, 2, 3)):
                    for tt, ps in qkv_proj(w_dram, brow_idx):
                        raw = apool.tile([128, D], BF16, tag="qkraw")
                        nc.scalar.activation(out=raw, in_=ps, func=AF.Copy)
                        rot = apool.tile([128, D], BF16, tag="qkrot")
                        rv = rot.rearrange("p (h x j) -> p h x j", h=H, x=2)
                        qv = raw.rearrange("p (h x j) -> p h x j", h=H, x=2)
                        cos_t = _bcast_heads(rope_sb[:, tt, tblc, :])
                        cos_t = bass.AP(cos_t.tensor, cos_t.offset,
                                        cos_t.ap[:2] + [[32, 2], [1, 32]])
                        sin_e = _bcast_heads(rope_sb[:, tt, tbls, 0:32])
                        sin_o = _bcast_heads(rope_sb[:, tt, tbls, 32:64])
                        tmp = apool.tile([128, D], BF16, tag="qktmp")
                        tv = tmp.rearrange("p (h x j) -> p h x j", h=H, x=2)
                        # tmp = swap_halves(q) * (+-sin)
                        nc.vector.tensor_mul(out=tv[:, :, 0, :],
                                             in0=qv[:, :, 1, :], in1=sin_e)
                        nc.vector.tensor_mul(out=tv[:, :, 1, :],
                                             in0=qv[:, :, 0, :], in1=sin_o)
                        nc.vector.tensor_mul(out=rv, in0=qv, in1=cos_t)
                        nc.vector.tensor_add(out=rot, in0=rot, in1=tmp)
                        for c in range(ND):
                            pt = psB.tile([128, 128], BF16, tag="B")
                            nc.tensor.transpose(out=pt, in_=rot[:, ts(c, 128)],
                                                identity=ident)
                            if c % 2 == 0:
                                nc.scalar.activation(
                                    out=dstT[:, c, ts(tt, 128)], in_=pt,
                                    func=AF.Copy)
                            else:
                                nc.vector.tensor_copy(
                                    out=dstT[:, c, ts(tt, 128)], in_=pt)

                # -- V (plain copy; ln1_b contribution folded into b_proj)
                Vs = slabs.tile([128, NT, D], BF16, tag="slab")
                for tt, ps in qkv_proj(wv_d, None):
                    nc.scalar.activation(out=Vs[:, tt, :], in_=ps, func=AF.Copy)

                # -- attention: 2 head-pairs (4 heads) per group; the two
                # pairs share one 2-bank denominator tile (ln/exp run once)
                oT = slabs.tile([128, ND, T], BF16, tag="slab")
                for qb in range(2):
                    n_sc = 4 * (qb + 1)
                    qsl = slice(qb * 512, (qb + 1) * 512)
                    for cg in range(ND // 2):       # cidx pair (2cg, 2cg+1)
                        dp2 = psA.tile([128, 1024], F32, tag="A")
                        pos = []
                        for ci in range(2):
                            cidx = 2 * cg + ci
                            h0 = 2 * cidx
                            kT0 = KT[0:64, cidx, :]
                            kT1 = KT[64:128, cidx, :]
                            qT0 = QT[0:64, cidx, qsl]
                            qT1 = QT[64:128, cidx, qsl]
                            pt0, pt1 = [], []
                            for spr in range(n_sc // 2):
                                sc0, sc1 = 2 * spr, 2 * spr + 1
                                for kT_h, qT_h, plist in ((kT0, qT0, pt0),
                                                          (kT1, qT1, pt1)):
                                    ps = psA.tile([128, 1024], F32, tag="A")
                                    nc.tensor.matmul(ps[:, 0:512],
                                                     kT_h[:, ts(sc0, 128)],
                                                     qT_h,
                                                     start=True, stop=True)
                                    nc.tensor.matmul(ps[:, 512:1024],
                                                     kT_h[:, ts(sc1, 128)],
                                                     qT_h,
                                                     start=True, stop=True)
                                    P = ppool.tile([128, 1024], BF16, tag="P")
                                    nc.scalar.activation(out=P, in_=ps,
                                                         func=AF.Exp)
                                    if spr >= 2 * qb:   # diagonal-crossing
                                        nc.gpsimd.tensor_mul(
                                            out=P, in0=P,
                                            in1=mask_sb[:, spr - 2 * qb, :])
                                    plist.append(P)
                            po = psB.tile([128, 512], F32, tag="B")
                            pos.append(po)
                            dsl = slice(ci * 512, ci * 512 + 512)
                            for sc in range(n_sc):
                                st, sp = (sc == 0), (sc == n_sc - 1)
                                o0 = (sc % 2) * 512
                                P0 = pt0[sc // 2][:, o0:o0 + 512]
                                P1 = pt1[sc // 2][:, o0:o0 + 512]
                                nc.tensor.matmul(
                                    po[0:64, :],
                                    Vs[:, sc, h0 * 64:h0 * 64 + 64],
                                    P0, start=st, stop=sp,
                                    tile_position=(0, 0))
                                nc.tensor.matmul(
                                    po[64:128, :],
                                    Vs[:, sc, h0 * 64 + 64:h0 * 64 + 128],
                                    P1, start=st, stop=sp,
                                    tile_position=(0, 64))
                                nc.tensor.matmul(dp2[0:64, dsl], ones64, P0,
                                                 start=st, stop=sp,
                                                 tile_position=(0, 0))
                                nc.tensor.matmul(dp2[64:128, dsl], ones64, P1,
                                                 start=st, stop=sp,
                                                 tile_position=(0, 64))
                        lnd = apool.tile([128, 1024], F32, tag="lnd")
                        nc.scalar.activation(out=lnd, in_=dp2, func=AF.Ln)
                        rec = apool.tile([128, 1024], BF16, tag="rec")
                        nc.scalar.activation(out=rec, in_=lnd, func=AF.Exp,
                                             scale=-1.0)
                        for ci in range(2):
                            cidx = 2 * cg + ci
                            nc.vector.tensor_mul(
                                out=oT[:, cidx, qsl], in0=pos[ci],
                                in1=rec[:, ci * 512:ci * 512 + 512])

                # -- proj (transposed out) + residual -> x2T (bf16)
                wp_sb = apool.tile([128, ND, D], BF16, tag="w")
                for wc in range(4):
                    nc.sync.dma_start(out=wp_sb[:, 2 * wc:2 * wc + 2, :],
                                      in_=wp_d.ap()[:, 2 * wc:2 * wc + 2, :])
                x2T = slabs.tile([128, ND, T], BF16, tag="slab")
                for dt in range(ND):
                    ps = psA.tile([128, 1024], F32, tag="A")
                    last = ND - 1
                    for c in range(ND):
                        fin = (c == last and not with_bias)
                        lhsT = wp_sb[:, c, ts(dt, 128)]
                        nc.tensor.matmul(ps[:, 0:512], lhsT,
                                         oT[:, c, 0:512],
                                         start=(c == 0), stop=fin)
                        nc.tensor.matmul(ps[:, 512:1024], lhsT,
                                         oT[:, c, 512:1024],
                                         start=(c == 0), stop=fin)
                    if with_bias:
                        bp = brows[0:1,
                                   2 * D + dt * 128:2 * D + dt * 128 + 128]
                        nc.tensor.matmul(ps[:, 0:512], bp, onesrow,
                                         start=False, stop=True)
                        nc.tensor.matmul(ps[:, 512:1024], bp, onesrow,
                                         start=False, stop=True)
                    nc.vector.tensor_add(out=x2T[:, dt, :], in0=ps,
                                         in1=xT[:, dt, :])

            # ---- LN2 ------------------------------------------------------
            z2T = slabs.tile([128, ND, T], BF16, tag="slab")
            layernorm_T(x2T, z2T)

            # ============ FFN super-phase (scoped pool) ====================
            fctx = contextlib.ExitStack()
            with fctx:
                fpool = fctx.enter_context(tc.tile_pool(name="fpool", bufs=1))
                w1pool = fctx.enter_context(tc.tile_pool(name="w1pool", bufs=2))
                w2pool = fctx.enter_context(tc.tile_pool(name="w2pool", bufs=2))
                opool = fctx.enter_context(tc.tile_pool(name="opool", bufs=4))
                for tb in range(2):
                    tbs = slice(tb * 512, (tb + 1) * 512)
                    # FFN1 half: hT[f, t-half] = relu(w1.T @ z2T + b1) on DVE
                    hTh = fpool.tile([128, NF, 512], BF16, tag="hTh")
                    for mg in range(NF // 4):
                        w1g = w1pool.tile([128, ND, 512], BF16, tag="w1g")
                        nc.sync.dma_start(
                            out=w1g,
                            in_=w1_d.ap()[:, :, mg * 512:(mg + 1) * 512]
                            .rearrange("c p f -> p c f"))
                        for mi in range(4):
                            m = mg * 4 + mi
                            ps = psB.tile([128, 512], F32, tag="B")
                            for c in range(ND):
                                nc.tensor.matmul(
                                    ps, w1g[:, c, ts(mi, 128)],
                                    z2T[:, c, tbs],
                                    start=(c == 0), stop=(c == ND - 1))
                            nc.vector.tensor_scalar(
                                out=hTh[:, m, :], in0=ps,
                                scalar1=b1t[:, m:m + 1], scalar2=0.0,
                                op0=ALU.add, op1=ALU.max)
                    # FFN2 half (transposed out) + residual -> out
                    for dt in range(ND):
                        w2c = w2pool.tile([128, NF, 128], BF16, tag="w2c")
                        nc.sync.dma_start(
                            out=w2c,
                            in_=w2_d.ap()[:, :, ts(dt, 128)]
                            .rearrange("c p f -> p c f"))
                        ps = psB.tile([128, 512], F32, tag="B")
                        last = NF - 1
                        for fc in range(NF):
                            fin = (fc == last and not with_bias)
                            nc.tensor.matmul(ps, w2c[:, fc, :],
                                             hTh[:, fc, :],
                                             start=(fc == 0), stop=fin)
                        if with_bias:
                            b2s = brows[0:1, 3 * D + dt * 128:
                                        3 * D + dt * 128 + 128]
                            nc.tensor.matmul(ps, b2s, onesrow,
                                             start=False, stop=True)
                        ot = opool.tile([128, 512], F32, tag="ot")
                        nc.vector.tensor_add(out=ot, in0=ps,
                                             in1=x2T[:, dt, tbs])
                        nc.sync.dma_start(out=out_d.ap()[:, dt, tbs], in_=ot)

    nc.compile()
    return nc


def _prep_inputs(inputs):
    """Host-side preprocessing: fold LN affine, permute rope cols, cast bf16."""
    f32 = np.float32
    x = np.asarray(inputs["x"], f32)
    wq = np.asarray(inputs["wq"], f32)
    wk = np.asarray(inputs["wk"], f32)
    wv = np.asarray(inputs["wv"], f32)
    w_proj = np.asarray(inputs["w_proj"], f32)
    b_proj = np.asarray(inputs["b_proj"], f32)
    ln1_w = np.asarray(inputs["ln1_w"], f32)
    ln1_b = np.asarray(inputs["ln1_b"], f32)
    ln2_w = np.asarray(inputs["ln2_w"], f32)
    ln2_b = np.asarray(inputs["ln2_b"], f32)
    w1 = np.asarray(inputs["w1"], f32)
    b1 = np.asarray(inputs["b1"], f32)
    w2 = np.asarray(inputs["w2"], f32)
    b2 = np.asarray(inputs["b2"], f32)

    bf = ml_dtypes.bfloat16
    perm = np.concatenate([np.arange(0, HS, 2), np.arange(1, HS, 2)])
    idx = (np.arange(H)[:, None] * HS + perm[None, :]).reshape(-1)

    wq_flat = wq.transpose(1, 0, 2).reshape(D, H * HS)
    wk_flat = wk.transpose(1, 0, 2).reshape(D, H * HS)
    wv_flat = wv.transpose(1, 0, 2).reshape(D, H * HS)
    wq_p = wq_flat[:, idx]
    wk_p = wk_flat[:, idx]

    def wlayout(w):  # [rows, cols] -> [128, ND, cols]  (p=row_in, c=row_chunk)
        return np.ascontiguousarray(
            w.reshape(ND, 128, D).transpose(1, 0, 2)).astype(bf)

    wq_h = wlayout(ln1_w[:, None] * wq_p)
    wk_h = wlayout(ln1_w[:, None] * wk_p)
    wv_h = wlayout(ln1_w[:, None] * wv_flat)
    wp_h = wlayout(w_proj)
    w1_h = np.ascontiguousarray(
        (ln2_w[:, None] * w1).reshape(ND, 128, F)).astype(bf)
    w2_h = np.ascontiguousarray(w2.reshape(NF, 128, D)).astype(bf)

    bq = ln1_b @ wq_p
    bk = ln1_b @ wk_p
    bv = ln1_b @ wv_flat
    bproj_eff = b_proj + bv @ w_proj
    b1_eff = ln2_b @ w1 + b1
    brows = np.concatenate([bq, bk, bproj_eff, b2]).reshape(1, 4 * D).astype(bf)
    b1t = np.ascontiguousarray(b1_eff.reshape(NF, 128).T).astype(f32)
    with_bias = bool(np.any(brows.astype(f32) != 0.0))

    # rope tables: [128, NT, 4, HS]; 4 = (cos_q, sin_q, cos_k, sin_k)
    t = np.arange(T, dtype=f32)
    th = (1.0 / 10000.0 ** (np.arange(0, HS, 2, dtype=f32) / f32(HS))).astype(f32)
    ang = t[:, None] * th[None, :]
    cos = np.concatenate([np.cos(ang), np.cos(ang)], 1)           # [T, HS]
    sin = np.concatenate([-np.sin(ang), np.sin(ang)], 1)
    sc = f32(HS) ** f32(-0.5)
    rope = np.stack([cos * sc, sin * sc, cos, sin], 1)            # [T, 4, HS]
    rope_h = np.ascontiguousarray(
        rope.reshape(NT, 128, 4, HS).transpose(1, 0, 2, 3)).astype(bf)

    # causal 0/1 pair-masks: pair 0 = s-tiles (j=0, j=1), pair 1 = (j=2, j=3)
    sl = np.arange(128)[:, None]
    ql = np.arange(512)[None, :]
    m4 = [(j * 128 + sl <= ql).astype(bf) for j in range(4)]
    mask_h = np.stack([np.concatenate([m4[0], m4[1]], 1),
                       np.concatenate([m4[2], m4[3]], 1)])        # [2, 128, 1024]
    mask_h = np.ascontiguousarray(mask_h.transpose(1, 0, 2))      # [128, 2, 1024]

    common = {
        "wq": wq_h, "wk": wk_h, "wv": wv_h, "wp": wp_h,
        "w1": w1_h, "w2": w2_h,
        "rope": rope_h, "mask": mask_h,
        "ident": np.eye(128, dtype=bf),
        "ones64": np.ones((128, 64), bf),
        "ones128": np.ones((128, 128), bf),
        "onesrow": np.ones((1, 512), bf),
        "brows": brows, "b1t": b1t,
    }
    in_maps = []
    for b in range(B):
        xTb = np.ascontiguousarray(
            x[b].T.reshape(ND, 128, T).transpose(1, 0, 2)).astype(bf)
        in_maps.append(dict(common, xt=xTb))
    return in_maps, with_bias


_NC_CACHE = {}


def get_nc(with_bias=False):
    key = ("nc", with_bias)
    if key not in _NC_CACHE:
        _NC_CACHE[key] = build_kernel(with_bias)
    return _NC_CACHE[key]


def _unpack(res):
    """results 'out' [128, ND, T] f32 -> stacked [B, T, D]."""
    outs = []
    for i in range(NCORES):
        o = res.results[i]["out"]                  # [128, ND, T]
        oT = o.transpose(1, 0, 2).reshape(D, T)    # [D, T]
        outs.append(np.ascontiguousarray(oT.T))    # [T, D]
    return np.stack(outs)


def kernel(**inputs):
    in_maps, with_bias = _prep_inputs(inputs)
    nc = get_nc(with_bias)
    res = run_bass_kernel_spmd(nc, in_maps, core_ids=list(range(NCORES)))
    return _unpack(res).astype(np.float32)


# revision 12
# speedup vs baseline: 1.0985x; 1.0985x over previous
"""Trainium2 Bass kernel for a dense transformer block (pre-LN, causal, RoPE).

Sharding: data-parallel over batch. B=8 batch elements, 8 NeuronCores, one
batch element per core; weights replicated. No collectives needed.

All-transposed dataflow (token axis T on the free dim everywhere) so no
on-device transposes are needed except Q/K (PE-transpose):
  xT [D, T] bf16 in; LN stats via ones-matmul (mean & E[x^2] land PSUM-
  replicated across partitions); znT = (xT - mu) * rstd directly [d, t].
  Q/K = znT.T @ wq/wk in [t, ch], RoPE (host tables, hs cols permuted
  even|odd, HS**-0.5 folded into q tables), PE-transpose -> QT/KT [ch, t].
  V = znT.T @ wv stays [t, ch].
  attention in head PAIRS via tile_position col-tiling; scoresT pairs share
  a 2-bank PSUM tile (one exp per [128,1024] on ACT); causal 0/1 masks
  multiply on GPSIMD; ones-matmuls write softmax denominators; normalize
  via exp(-ln(d)) (single ACT table set for the whole kernel - the
  act-table monkey-patch forces exp+ln onto natural_log_exp_and_others).
  proj outputs transposed [d, t] (lhsT = w_proj rows) + xT residual ->
  x2T; LN2 same as LN1; FFN1 -> hT [f, t] with relu+bias on DVE; FFN2
  outputs [d, t] (lhsT = w2 cols) + x2T residual -> out [D, T]; host
  transposes back to [T, D].  LN affine folded into weights host-side;
  bias matmuls are emitted only when some folded bias is nonzero.
"""

import os
import sys
import numpy as np

for _p in ("/opt/trn_rl_repo", "/root/.axon_site/_ro/trn_rl_repo"):
    if os.path.isdir(_p) and _p not in sys.path:
        sys.path.append(_p)

import ml_dtypes

import concourse.bass as bass
import concourse.tile as tile
from concourse import bacc, mybir
from concourse.bass import ts
from concourse.bass_utils import run_bass_kernel_spmd

BF16 = mybir.dt.bfloat16
F32 = mybir.dt.float32
AF = mybir.ActivationFunctionType
ALU = mybir.AluOpType

B, T, D, H, HS, F = 8, 1024, 1024, 16, 64, 4096
NT = T // 128   # 8 T-tiles
ND = D // 128   # 8 D-chunks
NF = F // 128   # 32 F-chunks
NCORES = 8


def _bcast_heads(ap2d, nheads=H):
    """[128, J] AP -> [128, nheads, J] broadcast along a step-0 middle dim."""
    return bass.AP(ap2d.tensor, ap2d.offset, [ap2d.ap[0], [0, nheads], ap2d.ap[-1]])


def _patch_act_tables():
    """Force Exp and Ln onto the combined natural_log_exp_and_others set so
    the whole kernel runs on ONE resident ACT table (no 1.3us reloads)."""
    from concourse import hw_specs, bacc as _bacc
    if getattr(hw_specs, "_act_tables_patched", False):
        return
    orig = hw_specs.get_activation_tables

    def patched(arch):
        t = orig(arch)
        if "natural_log_exp_and_others" in t:
            for name in ("exp_and_others", "natural_log", "exp_and_friends"):
                if name in t:
                    t[name] = set()
        return t

    hw_specs.get_activation_tables = patched
    _bacc.get_activation_tables = patched
    hw_specs._act_tables_patched = True


def build_kernel(with_bias=False):
    import contextlib

    _patch_act_tables()
    nc = bacc.Bacc("TRN2", target_bir_lowering=False, debug=False,
                   num_devices=NCORES)

    # ---- external I/O ------------------------------------------------------
    xt_d = nc.dram_tensor("xt", [128, ND, T], BF16, kind="ExternalInput")
    wq_d = nc.dram_tensor("wq", [128, ND, D], BF16, kind="ExternalInput")
    wk_d = nc.dram_tensor("wk", [128, ND, D], BF16, kind="ExternalInput")
    wv_d = nc.dram_tensor("wv", [128, ND, D], BF16, kind="ExternalInput")
    wp_d = nc.dram_tensor("wp", [128, ND, D], BF16, kind="ExternalInput")
    w1_d = nc.dram_tensor("w1", [ND, 128, F], BF16, kind="ExternalInput")
    w2_d = nc.dram_tensor("w2", [NF, 128, D], BF16, kind="ExternalInput")
    rope_d = nc.dram_tensor("rope", [128, NT, 4, HS], BF16, kind="ExternalInput")
    mask_d = nc.dram_tensor("mask", [128, 2, 1024], BF16, kind="ExternalInput")
    ident_d = nc.dram_tensor("ident", [128, 128], BF16, kind="ExternalInput")
    ones64_d = nc.dram_tensor("ones64", [128, 64], BF16, kind="ExternalInput")
    ones128_d = nc.dram_tensor("ones128", [128, 128], BF16, kind="ExternalInput")
    onesrow_d = nc.dram_tensor("onesrow", [1, 512], BF16, kind="ExternalInput")
    brows_d = nc.dram_tensor("brows", [1, 4 * D], BF16, kind="ExternalInput")
    b1t_d = nc.dram_tensor("b1t", [128, NF], F32, kind="ExternalInput")
    out_d = nc.dram_tensor("out", [128, ND, T], F32, kind="ExternalOutput")

    with tile.TileContext(nc) as tc:
        ctx = contextlib.ExitStack()
        with ctx:
            consts = ctx.enter_context(tc.tile_pool(name="consts", bufs=1))
            slabs = ctx.enter_context(tc.tile_pool(name="slabs", bufs=4))
            xpool = ctx.enter_context(tc.tile_pool(name="xpool", bufs=1))
            small = ctx.enter_context(tc.tile_pool(name="small", bufs=1))
            psA = ctx.enter_context(  # 2-bank tiles
                tc.tile_pool(name="psA", bufs=3, space="PSUM"))
            psB = ctx.enter_context(  # 1-bank tiles
                tc.tile_pool(name="psB", bufs=2, space="PSUM"))

            # ---- global constants -----------------------------------------
            ident = consts.tile([128, 128], BF16)
            nc.sync.dma_start(out=ident, in_=ident_d.ap())
            ones64 = consts.tile([128, 64], BF16)
            nc.sync.dma_start(out=ones64, in_=ones64_d.ap())
            ones128 = consts.tile([128, 128], BF16)
            nc.sync.dma_start(out=ones128, in_=ones128_d.ap())
            onesrow = consts.tile([1, 512], BF16)
            nc.sync.dma_start(out=onesrow, in_=onesrow_d.ap())
            b1t = consts.tile([128, NF], F32)
            nc.sync.dma_start(out=b1t, in_=b1t_d.ap())
            eps = consts.tile([128, 1], F32)
            nc.vector.memset(eps, 1e-5)
            brows = None
            if with_bias:
                brows = consts.tile([1, 4 * D], BF16)
                nc.sync.dma_start(out=brows, in_=brows_d.ap())

            # ---- x in (transposed, bf16) ----------------------------------
            xT = xpool.tile([128, ND, T], BF16)
            for xc in range(4):
                nc.sync.dma_start(out=xT[:, 2 * xc:2 * xc + 2, :],
                                  in_=xt_d.ap()[:, 2 * xc:2 * xc + 2, :])

            # ---- layernorm on transposed activations ----------------------
            # mean/E[x^2] via ones-matmul (PSUM-replicated over partitions);
            # rstd = exp(-0.5*ln(var+eps)); apply with two TT ops per chunk.
            def layernorm_T(src, dst):
                """src [128, ND, T] bf16 -> dst [128, ND, T] bf16."""
                mu_ps = psA.tile([128, 1024], F32, tag="A")
                ms_ps = psA.tile([128, 1024], F32, tag="A")
                for c in range(ND):
                    sq = small.tile([128, T], BF16, tag="sq")
                    nc.vector.tensor_mul(out=sq, in0=src[:, c, :],
                                         in1=src[:, c, :])
                    for tb in range(2):
                        tbs = slice(tb * 512, (tb + 1) * 512)
                        nc.tensor.matmul(mu_ps[:, tbs], ones128,
                                         src[:, c, tbs], start=(c == 0),
                                         stop=(c == ND - 1))
                        nc.tensor.matmul(ms_ps[:, tbs], ones128, sq[:, tbs],
                                         start=(c == 0), stop=(c == ND - 1))
                mu = small.tile([128, T], F32, tag="mu")
                nc.vector.tensor_scalar_mul(out=mu, in0=mu_ps,
                                            scalar1=1.0 / D)
                var = small.tile([128, T], F32, tag="var")
                nc.vector.tensor_mul(out=var, in0=mu, in1=mu)  # mu^2
                msd = small.tile([128, T], F32, tag="msd")
                nc.vector.tensor_scalar_mul(out=msd, in0=ms_ps,
                                            scalar1=1.0 / D)
                nc.vector.tensor_sub(out=var, in0=msd, in1=var)
                nc.scalar.activation(out=var, in_=var, func=AF.Ln,
                                     bias=eps, scale=1.0)
                rstd = small.tile([128, T], BF16, tag="rstd")
                nc.scalar.activation(out=rstd, in_=var, func=AF.Exp,
                                     scale=-0.5)
                for c in range(ND):
                    cen = small.tile([128, T], BF16, tag="cen")
                    nc.vector.tensor_sub(out=cen, in0=src[:, c, :], in1=mu)
                    nc.vector.tensor_mul(out=dst[:, c, :], in0=cen, in1=rstd)

            znT = slabs.tile([128, ND, T], BF16, tag="slab")
            layernorm_T(xT, znT)

            QT = slabs.tile([128, ND, T], BF16, tag="slab")
            KT = slabs.tile([128, ND, T], BF16, tag="slab")

            # ============ attention super-phase (scoped pool) ==============
            actx = contextlib.ExitStack()
            with actx:
                apool = actx.enter_context(tc.tile_pool(name="apool", bufs=2))
                ppool = actx.enter_context(tc.tile_pool(name="ppool", bufs=8))

                rope_sb = apool.tile([128, NT, 4, HS], BF16, tag="rope")
                nc.sync.dma_start(out=rope_sb, in_=rope_d.ap())
                mask_sb = apool.tile([128, 2, 1024], BF16, tag="mask")
                nc.sync.dma_start(out=mask_sb, in_=mask_d.ap())

                def qkv_proj(w_dram, brow_idx):
                    w_sb = apool.tile([128, ND, D], BF16, tag="w")
                    for wc in range(4):
                        nc.sync.dma_start(
                            out=w_sb[:, 2 * wc:2 * wc + 2, :],
                            in_=w_dram.ap()[:, 2 * wc:2 * wc + 2, :])
                    for tt in range(NT):
                        ps = psA.tile([128, 1024], F32, tag="A")
                        last = ND - 1
                        for c in range(ND):
                            fin = (c == last and brow_idx is None)
                            lhsT = znT[:, c, ts(tt, 128)]
                            nc.tensor.matmul(ps[:, 0:512], lhsT,
                                             w_sb[:, c, 0:512],
                                             start=(c == 0), stop=fin)
                            nc.tensor.matmul(ps[:, 512:1024], lhsT,
                                             w_sb[:, c, 512:1024],
                                             start=(c == 0), stop=fin)
                        if brow_idx is not None:
                            o = brow_idx * D
                            nc.tensor.matmul(ps[:, 0:512], onesrow[:, 0:128],
                                             brows[0:1, o:o + 512],
                                             start=False, stop=True)
                            nc.tensor.matmul(ps[:, 512:1024],
                                             onesrow[:, 0:128],
                                             brows[0:1, o + 512:o + 1024],
                                             start=False, stop=True)
                        yield tt, ps

                # -- Q then K: copy out of PSUM, rope, PE-transpose
                for w_dram, brow_idx, dstT, tblc, tbls in (
                        (wq_d, 0 if with_bias else None, QT, 0, 1),
                        (wk_d, 1 if with_bias else None, KT, 2, 3)):
                    for tt, ps in qkv_proj(w_dram, brow_idx):
                        raw = apool.tile([128, D], BF16, tag="qkraw")
                        nc.scalar.activation(out=raw, in_=ps, func=AF.Copy)
                        rot = apool.tile([128, D], BF16, tag="qkrot")
                        rv = rot.rearrange("p (h x j) -> p h x j", h=H, x=2)
                        qv = raw.rearrange("p (h x j) -> p h x j", h=H, x=2)
                        cos_t = _bcast_heads(rope_sb[:, tt, tblc, :])
                        cos_t = bass.AP(cos_t.tensor, cos_t.offset,
                                        cos_t.ap[:2] + [[32, 2], [1, 32]])
                        sin_e = _bcast_heads(rope_sb[:, tt, tbls, 0:32])
                        sin_o = _bcast_heads(rope_sb[:, tt, tbls, 32:64])
                        tmp = apool.tile([128, D], BF16, tag="qktmp")
                        tv = tmp.rearrange("p (h x j) -> p h x j", h=H, x=2)
                        # tmp = swap_halves(q) * (+-sin)
                        nc.vector.tensor_mul(out=tv[:, :, 0, :],
                                             in0=qv[:, :, 1, :], in1=sin_e)
                        nc.vector.tensor_mul(out=tv[:, :, 1, :],
                                             in0=qv[:, :, 0, :], in1=sin_o)
                        nc.vector.tensor_mul(out=rv, in0=qv, in1=cos_t)
                        nc.vector.tensor_add(out=rot, in0=rot, in1=tmp)
                        for c in range(ND):
                            pt = psB.tile([128, 128], BF16, tag="B")
                            nc.tensor.transpose(out=pt, in_=rot[:, ts(c, 128)],
                                                identity=ident)
                            if c % 2 == 0:
                                nc.scalar.activation(
                                    out=dstT[:, c, ts(tt, 128)], in_=pt,
                                    func=AF.Copy)
                            else:
                                nc.vector.tensor_copy(
                                    out=dstT[:, c, ts(tt, 128)], in_=pt)

                # -- V (plain copy; ln1_b contribution folded into b_proj)
                Vs = slabs.tile([128, NT, D], BF16, tag="slab")
                for tt, ps in qkv_proj(wv_d, None):
                    nc.scalar.activation(out=Vs[:, tt, :], in_=ps, func=AF.Copy)

                # -- attention: 2 head-pairs (4 heads) per group; the two
                # pairs share one 2-bank denominator tile (ln/exp run once)
                oT = slabs.tile([128, ND, T], BF16, tag="slab")
                for qb in range(2):
                    n_sc = 4 * (qb + 1)
                    qsl = slice(qb * 512, (qb + 1) * 512)
                    for cg in range(ND // 2):       # cidx pair (2cg, 2cg+1)
                        dp2 = psA.tile([128, 1024], F32, tag="A")
                        pos = []
                        for ci in range(2):
                            cidx = 2 * cg + ci
                            h0 = 2 * cidx
                            kT0 = KT[0:64, cidx, :]
                            kT1 = KT[64:128, cidx, :]
                            qT0 = QT[0:64, cidx, qsl]
                            qT1 = QT[64:128, cidx, qsl]
                            pt0, pt1 = [], []
                            for spr in range(n_sc // 2):
                                sc0, sc1 = 2 * spr, 2 * spr + 1
                                for kT_h, qT_h, plist in ((kT0, qT0, pt0),
                                                          (kT1, qT1, pt1)):
                                    ps = psA.tile([128, 1024], F32, tag="A")
                                    nc.tensor.matmul(ps[:, 0:512],
                                                     kT_h[:, ts(sc0, 128)],
                                                     qT_h,
                                                     start=True, stop=True)
                                    nc.tensor.matmul(ps[:, 512:1024],
                                                     kT_h[:, ts(sc1, 128)],
                                                     qT_h,
                                                     start=True, stop=True)
                                    P = ppool.tile([128, 1024], BF16, tag="P")
                                    nc.scalar.activation(out=P, in_=ps,
                                                         func=AF.Exp)
                                    if spr >= 2 * qb:   # diagonal-crossing
                                        nc.vector.tensor_mul(
                                            out=P, in0=P,
                                            in1=mask_sb[:, spr - 2 * qb, :])
                                    plist.append(P)
                            po = psB.tile([128, 512], F32, tag="B")
                            pos.append(po)
                            dsl = slice(ci * 512, ci * 512 + 512)
                            for sc in range(n_sc):
                                st, sp = (sc == 0), (sc == n_sc - 1)
                                o0 = (sc % 2) * 512
                                P0 = pt0[sc // 2][:, o0:o0 + 512]
                                P1 = pt1[sc // 2][:, o0:o0 + 512]
                                nc.tensor.matmul(
                                    po[0:64, :],
                                    Vs[:, sc, h0 * 64:h0 * 64 + 64],
                                    P0, start=st, stop=sp,
                                    tile_position=(0, 0))
                                nc.tensor.matmul(
                                    po[64:128, :],
                                    Vs[:, sc, h0 * 64 + 64:h0 * 64 + 128],
                                    P1, start=st, stop=sp,
                                    tile_position=(0, 64))
                                nc.tensor.matmul(dp2[0:64, dsl], ones64, P0,
                                                 start=st, stop=sp,
                                                 tile_position=(0, 0))
                                nc.tensor.matmul(dp2[64:128, dsl], ones64, P1,
                                                 start=st, stop=sp,
                                                 tile_position=(0, 64))
                        lnd = apool.tile([128, 1024], F32, tag="lnd")
                        nc.scalar.activation(out=lnd, in_=dp2, func=AF.Ln)
                        rec = apool.tile([128, 1024], BF16, tag="rec")
                        nc.scalar.activation(out=rec, in_=lnd, func=AF.Exp,
                                             scale=-1.0)
                        for ci in range(2):
                            cidx = 2 * cg + ci
                            nc.vector.tensor_mul(
                                out=oT[:, cidx, qsl], in0=pos[ci],
                                in1=rec[:, ci * 512:ci * 512 + 512])

                # -- proj (transposed out) + residual -> x2T (bf16)
                wp_sb = apool.tile([128, ND, D], BF16, tag="w")
                for wc in range(4):
                    nc.sync.dma_start(out=wp_sb[:, 2 * wc:2 * wc + 2, :],
                                      in_=wp_d.ap()[:, 2 * wc:2 * wc + 2, :])
                x2T = slabs.tile([128, ND, T], BF16, tag="slab")
                for dt in range(ND):
                    ps = psA.tile([128, 1024], F32, tag="A")
                    last = ND - 1
                    for c in range(ND):
                        fin = (c == last and not with_bias)
                        lhsT = wp_sb[:, c, ts(dt, 128)]
                        nc.tensor.matmul(ps[:, 0:512], lhsT,
                                         oT[:, c, 0:512],
                                         start=(c == 0), stop=fin)
                        nc.tensor.matmul(ps[:, 512:1024], lhsT,
                                         oT[:, c, 512:1024],
                                         start=(c == 0), stop=fin)
                    if with_bias:
                        bp = brows[0:1,
                                   2 * D + dt * 128:2 * D + dt * 128 + 128]
                        nc.tensor.matmul(ps[:, 0:512], bp, onesrow,
                                         start=False, stop=True)
                        nc.tensor.matmul(ps[:, 512:1024], bp, onesrow,
                                         start=False, stop=True)
                    nc.vector.tensor_add(out=x2T[:, dt, :], in0=ps,
                                         in1=xT[:, dt, :])

            # ============ FFN super-phase (scoped pool) ====================
            fctx = contextlib.ExitStack()
            with fctx:
                fpool = fctx.enter_context(tc.tile_pool(name="fpool", bufs=1))
                w1pool = fctx.enter_context(tc.tile_pool(name="w1pool", bufs=2))
                w2pool = fctx.enter_context(tc.tile_pool(name="w2pool", bufs=2))
                opool = fctx.enter_context(tc.tile_pool(name="opool", bufs=4))
                # prefetch the first w1 group while LN2 runs
                w1gs = {}
                for mg in range(2):
                    w1g = w1pool.tile([128, ND, 512], BF16, tag="w1g")
                    nc.sync.dma_start(
                        out=w1g,
                        in_=w1_d.ap()[:, :, mg * 512:(mg + 1) * 512]
                        .rearrange("c p f -> p c f"))
                    w1gs[mg] = w1g

                # ---- LN2 --------------------------------------------------
                z2T = slabs.tile([128, ND, T], BF16, tag="slab")
                layernorm_T(x2T, z2T)

                for tb in range(2):
                    tbs = slice(tb * 512, (tb + 1) * 512)
                    # FFN1 half: hT[f, t-half] = relu(w1.T @ z2T + b1) on DVE
                    hTh = fpool.tile([128, NF, 512], BF16, tag="hTh")
                    for mg in range(NF // 4):
                        if (tb, mg) in ((0, 0), (0, 1)):
                            w1g = w1gs.pop(mg)
                        else:
                            w1g = w1pool.tile([128, ND, 512], BF16, tag="w1g")
                            nc.sync.dma_start(
                                out=w1g,
                                in_=w1_d.ap()[:, :, mg * 512:(mg + 1) * 512]
                                .rearrange("c p f -> p c f"))
                        for mi in range(4):
                            m = mg * 4 + mi
                            ps = psB.tile([128, 512], F32, tag="B")
                            for c in range(ND):
                                nc.tensor.matmul(
                                    ps, w1g[:, c, ts(mi, 128)],
                                    z2T[:, c, tbs],
                                    start=(c == 0), stop=(c == ND - 1))
                            nc.vector.tensor_scalar(
                                out=hTh[:, m, :], in0=ps,
                                scalar1=b1t[:, m:m + 1], scalar2=0.0,
                                op0=ALU.add, op1=ALU.max)
                    # FFN2 half (transposed out) + residual -> out
                    for dt in range(ND):
                        w2c = w2pool.tile([128, NF, 128], BF16, tag="w2c")
                        nc.sync.dma_start(
                            out=w2c,
                            in_=w2_d.ap()[:, :, ts(dt, 128)]
                            .rearrange("c p f -> p c f"))
                        ps = psB.tile([128, 512], F32, tag="B")
                        last = NF - 1
                        for fc in range(NF):
                            fin = (fc == last and not with_bias)
                            nc.tensor.matmul(ps, w2c[:, fc, :],
                                             hTh[:, fc, :],
                                             start=(fc == 0), stop=fin)
                        if with_bias:
                            b2s = brows[0:1, 3 * D + dt * 128:
                                        3 * D + dt * 128 + 128]
                            nc.tensor.matmul(ps, b2s, onesrow,
                                             start=False, stop=True)
                        ot = opool.tile([128, 512], F32, tag="ot")
                        nc.vector.tensor_add(out=ot, in0=ps,
                                             in1=x2T[:, dt, tbs])
                        nc.sync.dma_start(out=out_d.ap()[:, dt, tbs], in_=ot)

    nc.compile()
    return nc


def _prep_inputs(inputs):
    """Host-side preprocessing: fold LN affine, permute rope cols, cast bf16."""
    f32 = np.float32
    x = np.asarray(inputs["x"], f32)
    wq = np.asarray(inputs["wq"], f32)
    wk = np.asarray(inputs["wk"], f32)
    wv = np.asarray(inputs["wv"], f32)
    w_proj = np.asarray(inputs["w_proj"], f32)
    b_proj = np.asarray(inputs["b_proj"], f32)
    ln1_w = np.asarray(inputs["ln1_w"], f32)
    ln1_b = np.asarray(inputs["ln1_b"], f32)
    ln2_w = np.asarray(inputs["ln2_w"], f32)
    ln2_b = np.asarray(inputs["ln2_b"], f32)
    w1 = np.asarray(inputs["w1"], f32)
    b1 = np.asarray(inputs["b1"], f32)
    w2 = np.asarray(inputs["w2"], f32)
    b2 = np.asarray(inputs["b2"], f32)

    bf = ml_dtypes.bfloat16
    perm = np.concatenate([np.arange(0, HS, 2), np.arange(1, HS, 2)])
    idx = (np.arange(H)[:, None] * HS + perm[None, :]).reshape(-1)

    wq_flat = wq.transpose(1, 0, 2).reshape(D, H * HS)
    wk_flat = wk.transpose(1, 0, 2).reshape(D, H * HS)
    wv_flat = wv.transpose(1, 0, 2).reshape(D, H * HS)
    wq_p = wq_flat[:, idx]
    wk_p = wk_flat[:, idx]

    def wlayout(w):  # [rows, cols] -> [128, ND, cols]  (p=row_in, c=row_chunk)
        return np.ascontiguousarray(
            w.reshape(ND, 128, D).transpose(1, 0, 2)).astype(bf)

    wq_h = wlayout(ln1_w[:, None] * wq_p)
    wk_h = wlayout(ln1_w[:, None] * wk_p)
    wv_h = wlayout(ln1_w[:, None] * wv_flat)
    wp_h = wlayout(w_proj)
    w1_h = np.ascontiguousarray(
        (ln2_w[:, None] * w1).reshape(ND, 128, F)).astype(bf)
    w2_h = np.ascontiguousarray(w2.reshape(NF, 128, D)).astype(bf)

    bq = ln1_b @ wq_p
    bk = ln1_b @ wk_p
    bv = ln1_b @ wv_flat
    bproj_eff = b_proj + bv @ w_proj
    b1_eff = ln2_b @ w1 + b1
    brows = np.concatenate([bq, bk, bproj_eff, b2]).reshape(1, 4 * D).astype(bf)
    b1t = np.ascontiguousarray(b1_eff.reshape(NF, 128).T).astype(f32)
    with_bias = bool(np.any(brows.astype(f32) != 0.0))

    # rope tables: [128, NT, 4, HS]; 4 = (cos_q, sin_q, cos_k, sin_k)
    t = np.arange(T, dtype=f32)
    th = (1.0 / 10000.0 ** (np.arange(0, HS, 2, dtype=f32) / f32(HS))).astype(f32)
    ang = t[:, None] * th[None, :]
    cos = np.concatenate([np.cos(ang), np.cos(ang)], 1)           # [T, HS]
    sin = np.concatenate([-np.sin(ang), np.sin(ang)], 1)
    sc = f32(HS) ** f32(-0.5)
    rope = np.stack([cos * sc, sin * sc, cos, sin], 1)            # [T, 4, HS]
    rope_h = np.ascontiguousarray(
        rope.reshape(NT, 128, 4, HS).transpose(1, 0, 2, 3)).astype(bf)

    # causal 0/1 pair-masks: pair 0 = s-tiles (j=0, j=1), pair 1 = (j=2, j=3)
    sl = np.arange(128)[:, None]
    ql = np.arange(512)[None, :]
    m4 = [(j * 128 + sl <= ql).astype(bf) for j in range(4)]
    mask_h = np.stack([np.concatenate([m4[0], m4[1]], 1),
                       np.concatenate([m4[2], m4[3]], 1)])        # [2, 128, 1024]
    mask_h = np.ascontiguousarray(mask_h.transpose(1, 0, 2))      # [128, 2, 1024]

    common = {
        "wq": wq_h, "wk": wk_h, "wv": wv_h, "wp": wp_h,
        "w1": w1_h, "w2": w2_h,
        "rope": rope_h, "mask": mask_h,
        "ident": np.eye(128, dtype=bf),
        "ones64": np.ones((128, 64), bf),
        "ones128": np.ones((128, 128), bf),
        "onesrow": np.ones((1, 512), bf),
        "brows": brows, "b1t": b1t,
    }
    in_maps = []
    for b in range(B):
        xTb = np.ascontiguousarray(
            x[b].T.reshape(ND, 128, T).transpose(1, 0, 2)).astype(bf)
        in_maps.append(dict(common, xt=xTb))
    return in_maps, with_bias


_NC_CACHE = {}


def get_nc(with_bias=False):
    key = ("nc", with_bias)
    if key not in _NC_CACHE:
        _NC_CACHE[key] = build_kernel(with_bias)
    return _NC_CACHE[key]


def _unpack(res):
    """results 'out' [128, ND, T] f32 -> stacked [B, T, D]."""
    outs = []
    for i in range(NCORES):
        o = res.results[i]["out"]                  # [128, ND, T]
        oT = o.transpose(1, 0, 2).reshape(D, T)    # [D, T]
        outs.append(np.ascontiguousarray(oT.T))    # [T, D]
    return np.stack(outs)


def kernel(**inputs):
    in_maps, with_bias = _prep_inputs(inputs)
    nc = get_nc(with_bias)
    res = run_bass_kernel_spmd(nc, in_maps, core_ids=list(range(NCORES)))
    return _unpack(res).astype(np.float32)
